# revision 1
# baseline (speedup 1.0000x reference)
"""Trainium2 Bass kernel for nn_Attn (dense_transformer).

Reference computation:
    proj     = einsum('sbh,oh->sbo', encoder_outputs, attn_W) + attn_b   # [S,B,H]
    energies = einsum('sbh,bh->bs', proj, hidden[0])                     # [B,S]
    out      = log_softmax(energies, axis=-1)[:, None, :]                # [B,1,S]

Algebraic rewrite used here:
    energies[b,s] = enc[s,b,:] . (W^T @ hidden[b]) + attn_b . hidden[b]
The per-b constant attn_b . hidden[b] cancels inside log_softmax, so the
kernel computes   log_softmax_s( enc[s,b,:] . v[b] )   with v = hidden @ W.
This turns a 137-GFLOP projection into a memory-bound streaming reduction
over the 256MB encoder tensor plus a tiny [32,1024]x[1024,1024] matvec.

Sharding: data-parallel over batch B=32 -> 4 batches per core on 8 cores.
Each core streams its contiguous 32MB slice of encoder_outputs, computes
v on-device from the replicated 4MB weight, reduces with a fused
multiply+accumulate (scalar_tensor_tensor) on the Vector engine, and does
the log-softmax entirely in the transposed [s1, (i,b)] accumulator layout
(cross-partition stats via gpsimd.partition_all_reduce), finishing with a
single PE transpose + one DMA to the output. No collectives needed.
"""

import numpy as np

S, B, H = 2048, 32, 1024
N_CORES = 8
B_LOC = B // N_CORES          # 4 batches per core
N_TILES = S // 128            # 16 s-tiles of 128 rows
F = B_LOC * H                 # 4096 free elements per s-row
ENC_BUFS = 10

_CACHE = {}


def _build():
    import concourse.bacc as bacc
    import concourse.bass_isa as bass_isa
    import concourse.mybir as mybir
    import concourse.tile as tile
    from concourse import masks
    from concourse.tile import add_dep_helper
    from contextlib import ExitStack

    f32 = mybir.dt.float32
    f16 = mybir.dt.float16
    nc = bacc.Bacc("TRN2", target_bir_lowering=False, debug=False,
                   num_devices=N_CORES)

    enc = nc.dram_tensor("enc", [S, F], f16, kind="ExternalInput").ap()
    hid = nc.dram_tensor("hid", [B_LOC, H], f16, kind="ExternalInput").ap()
    w = nc.dram_tensor("w", [H, H], f16, kind="ExternalInput").ap()
    out = nc.dram_tensor("out", [B_LOC, S], f32, kind="ExternalOutput").ap()

    with tile.TileContext(nc) as tc, ExitStack() as ctx:
        const_pool = ctx.enter_context(tc.tile_pool(name="const", bufs=1))
        w_pool = ctx.enter_context(tc.tile_pool(name="wpool", bufs=4))
        enc_pool = ctx.enter_context(tc.tile_pool(name="encp", bufs=ENC_BUFS))
        scr_pool = ctx.enter_context(tc.tile_pool(name="scr", bufs=5))
        ps_pool = ctx.enter_context(tc.tile_pool(name="ps", bufs=2, space="PSUM"))
        psw_pool = ctx.enter_context(tc.tile_pool(name="psw", bufs=1, space="PSUM"))
        psv_pool = ctx.enter_context(tc.tile_pool(name="psv", bufs=1, space="PSUM"))

        # ---- constants -------------------------------------------------
        identity = const_pool.tile([128, 128], f32)
        masks.make_identity(nc, identity[:])
        # sel[b, b*128:(b+1)*128] = 1 : one-hot rows used to broadcast v[b]
        # (band mask: partition-base-0 ops only).
        sel = const_pool.tile([B_LOC, B_LOC * 128], f32)
        nc.gpsimd.memset(sel[:], 1.0)
        nc.gpsimd.affine_select(
            out=sel[:], in_=sel[:], compare_op=mybir.AluOpType.is_ge,
            fill=0.0, base=0, pattern=[[1, B_LOC * 128]],
            channel_multiplier=-128)
        nc.gpsimd.affine_select(
            out=sel[:], in_=sel[:], compare_op=mybir.AluOpType.is_ge,
            fill=0.0, base=127, pattern=[[-1, B_LOC * 128]],
            channel_multiplier=128)

        # Preload the exp/ln ACT table sets while ScalarE is idle so the
        # epilogue doesn't pay the ~2.6us ACT_TABLE_LOAD cost.
        warm = const_pool.tile([1, 1], f32)
        nc.vector.memset(warm[:], 1.0)
        warm2 = const_pool.tile([1, 1], f32)
        nc.scalar.activation(warm2[:], warm[:], mybir.ActivationFunctionType.Exp)
        nc.scalar.activation(warm2[:], warm2[:], mybir.ActivationFunctionType.Ln)

        # ---- v = hid @ W  (v[b,h] = sum_o hid[b,o] W[o,h]) -------------
        hid_sb = const_pool.tile([B_LOC, H], f16)
        nc.sync.dma_start(hid_sb[:], hid[:, :])

        # transpose hid -> hidT[o_chunk][128, B_LOC]
        identity16 = const_pool.tile([B_LOC, B_LOC], f16)
        nc.vector.tensor_copy(identity16[:], identity[:B_LOC, :B_LOC])
        hidT = const_pool.tile([128, 8 * B_LOC], f16)
        for oc in range(8):
            pt = ps_pool.tile([128, B_LOC], f16, tag="mmt")
            nc.tensor.transpose(pt[:], hid_sb[:, oc * 128:(oc + 1) * 128],
                                identity16[:])
            nc.scalar.copy(hidT[:, oc * B_LOC:(oc + 1) * B_LOC], pt[:])

        w_tiles = []
        w_dmas = []
        for oc in range(8):
            wt = w_pool.tile([128, H], f16, tag="wt")
            w_dmas.append(nc.sync.dma_start(wt[:], w[oc * 128:(oc + 1) * 128, :]))
            w_tiles.append(wt)

        psum_v = psv_pool.tile([B_LOC, H], f32)
        for oc in range(8):
            for hc in range(2):
                nc.tensor.matmul(
                    psum_v[:, hc * 512:(hc + 1) * 512],
                    lhsT=hidT[:, oc * B_LOC:(oc + 1) * B_LOC],
                    rhs=w_tiles[oc][:, hc * 512:(hc + 1) * 512],
                    start=(oc == 0), stop=(oc == 7),
                    skip_group_check=True)
        v_sb = const_pool.tile([B_LOC, H], f16)
        nc.scalar.copy(v_sb[:], psum_v[:])
        sel16 = const_pool.tile([B_LOC, B_LOC * 128], f16)
        nc.vector.tensor_copy(sel16[:], sel[:])

        # ---- broadcast v across all 128 partitions ---------------------
        # vb[p, b*H + h] = v[b, h] for every partition p (fp16: the whole
        # per-tile multiply is a single 2x-mode DVE op against it)
        vb = const_pool.tile([128, F], f16)
        for b in range(B_LOC):
            for hc in range(2):
                pbc = ps_pool.tile([128, 512], f32, tag="mm")
                nc.tensor.matmul(pbc[:],
                                 lhsT=sel16[:, b * 128:(b + 1) * 128],
                                 rhs=v_sb[:, hc * 512:(hc + 1) * 512],
                                 start=True, stop=True)
                lo = b * H + hc * 512
                if (b + hc) % 2 == 0:
                    nc.scalar.copy(vb[:, lo:lo + 512], pbc[:])
                else:
                    nc.vector.tensor_copy(vb[:, lo:lo + 512], pbc[:])

        # ---- main loop: energies via fused multiply+reduce -------------
        # acc[s1, i*4+b] = sum_h enc[i*128+s1, b, h] * v[b, h]
        acc = const_pool.tile([128, N_TILES * B_LOC], f32)
        for i in range(N_TILES):
            et = enc_pool.tile([128, F], f16)
            enc_dma = nc.sync.dma_start(et[:], enc[i * 128:(i + 1) * 128, :])
            if i < ENC_BUFS:
                # Keep most of the DMA bandwidth on the critical-path weight
                # load: enc tile i only starts once W tile 3+i is in, so W
                # finishes ~2.5x sooner while the enc stream ramps without a
                # bandwidth bubble; the deep enc buffer then absorbs the
                # stream until the DVE starts consuming.
                add_dep_helper(enc_dma.ins, w_dmas[min(3 + i, 7)].ins,
                               reason="prioritize W stream over enc stream")
            # one fp16 2x-mode multiply covering all four b's, then the
            # per-b free-dim sums: three on ScalarE, one on DVE, so the
            # two engines run ~balanced (~3.6us each per tile).
            so = scr_pool.tile([128, F], f16, tag="so")
            nc.vector.tensor_mul(so[:], et[:], vb[:])
            for b in range(B_LOC):
                col = acc[:, i * B_LOC + b: i * B_LOC + b + 1]
                if b == 3:
                    so2 = scr_pool.tile([128, H], f16, tag="so2")
                    nc.vector.tensor_scalar(
                        so2[:], so[:, b * H:(b + 1) * H], 1.0, 0.0,
                        op0=mybir.AluOpType.mult,
                        op1=mybir.AluOpType.add,
                        accum_out=col)
                else:
                    so3 = scr_pool.tile([128, H], f16, tag="so3")
                    nc.scalar.activation(
                        so3[:], so[:, b * H:(b + 1) * H],
                        mybir.ActivationFunctionType.Copy,
                        bias=0.0, scale=1.0,
                        accum_out=col)

        # ---- log_softmax over s, computed in the [s1, (i,b)] layout ----
        # per-b max over i, then over partitions (same value lands on all
        # partitions, i.e. already broadcast for the subtraction APs)
        macc = const_pool.tile([128, B_LOC], f32)
        nc.vector.reduce_max(macc[:],
                             acc[:].rearrange("p (i b) -> p b i", b=B_LOC),
                             axis=mybir.AxisListType.X)
        nc.gpsimd.partition_all_reduce(macc[:], macc[:], 128,
                                       bass_isa.ReduceOp.max)
        sub = const_pool.tile([128, N_TILES * B_LOC], f32)
        nc.vector.tensor_tensor(
            out=sub[:].rearrange("p (i b) -> p i b", b=B_LOC),
            in0=acc[:].rearrange("p (i b) -> p i b", b=B_LOC),
            in1=macc[:, :].unsqueeze(1).broadcast_to([128, N_TILES, B_LOC]),
            op=mybir.AluOpType.subtract)
        pexp = const_pool.tile([128, N_TILES * B_LOC], f32)
        nc.scalar.activation(pexp[:], sub[:], mybir.ActivationFunctionType.Exp)
        ssum = const_pool.tile([128, B_LOC], f32)
        nc.vector.reduce_sum(ssum[:],
                             pexp[:].rearrange("p (i b) -> p b i", b=B_LOC),
                             axis=mybir.AxisListType.X)
        nc.gpsimd.partition_all_reduce(ssum[:], ssum[:], 128,
                                       bass_isa.ReduceOp.add)
        lse = const_pool.tile([128, B_LOC], f32)
        nc.scalar.activation(lse[:], ssum[:], mybir.ActivationFunctionType.Ln)
        # out = (acc - max) - ln(sum) = sub - lse
        outacc = const_pool.tile([128, N_TILES * B_LOC], f32)
        nc.vector.tensor_tensor(
            out=outacc[:].rearrange("p (i b) -> p i b", b=B_LOC),
            in0=sub[:].rearrange("p (i b) -> p i b", b=B_LOC),
            in1=lse[:, :].unsqueeze(1).broadcast_to([128, N_TILES, B_LOC]),
            op=mybir.AluOpType.subtract)

        # transpose [s1, (i,b)] -> [(i,b), s1] and DMA straight to out
        pe_ps = psw_pool.tile([N_TILES * B_LOC, 128], f32, tag="pswt")
        nc.tensor.transpose(pe_ps[:], outacc[:], identity[:])
        e_sb = const_pool.tile([N_TILES * B_LOC, 128], f32)
        nc.scalar.copy(e_sb[:], pe_ps[:])
        nc.sync.dma_start(out.rearrange("b (i s) -> i b s", i=N_TILES),
                          e_sb[:])

    nc.compile()
    return nc


def _get_nc():
    if "nc" not in _CACHE:
        _CACHE["nc"] = _build()
    return _CACHE["nc"]


def kernel(hidden, encoder_outputs, attn_W, attn_b):
    from concourse.bass_utils import run_bass_kernel_spmd

    hidden = np.asarray(hidden, dtype=np.float32)
    encoder_outputs = np.asarray(encoder_outputs, dtype=np.float32)
    attn_W = np.ascontiguousarray(np.asarray(attn_W, dtype=np.float16))

    in_maps = []
    for c in range(N_CORES):
        b0 = c * B_LOC
        enc_loc = np.ascontiguousarray(
            encoder_outputs[:, b0:b0 + B_LOC, :]).reshape(S, F).astype(np.float16)
        hid_loc = np.ascontiguousarray(
            hidden[0, b0:b0 + B_LOC, :]).astype(np.float16)
        in_maps.append({"enc": enc_loc, "hid": hid_loc, "w": attn_W})

    nc = _get_nc()
    res = run_bass_kernel_spmd(nc, in_maps, core_ids=list(range(N_CORES)))
    _CACHE["last_results"] = res
    outs = [r["out"] for r in res.results]          # each [B_LOC, S]
    full = np.concatenate(outs, axis=0)             # [B, S]
    return full[:, None, :].astype(np.float32)      # [B, 1, S]



# revision 13
# speedup vs baseline: 1.4737x; 1.4737x over previous
"""Trainium2 Bass kernel for nn_Attn (dense_transformer).

Reference computation:
    proj     = einsum('sbh,oh->sbo', encoder_outputs, attn_W) + attn_b   # [S,B,H]
    energies = einsum('sbh,bh->bs', proj, hidden[0])                     # [B,S]
    out      = log_softmax(energies, axis=-1)[:, None, :]                # [B,1,S]

Algebraic rewrite:
    energies[b,s] = enc[s,b,:] . v[b]  with  v = hidden @ W  (the attn_b
    term is constant per b and cancels inside log_softmax).

Kernel strategy (v2 — TensorE-centric):
  - Data-parallel over batch: 4 b's per core on 8 cores, no collectives.
  - enc is pre-transposed on the host to [h, s] layout and quantized to
    fp8-e3m4 (halves HBM traffic; measured end-to-end rel-err ~9e-3 vs
    the 2e-2 gate). W stays fp16 so v keeps full precision.
  - The PE computes vT = W^T @ hidT directly in [h, b] layout (64 small
    matmuls pipelined behind the W DMA chunks), then the entire
    energies reduction runs as 128 N=512 matmuls with vT stationary:
    out[b', s-block] += vT[hc]^T @ encT[hc, b, s-block], accumulated
    over the 8 h-chunks in PSUM. Row b of each [4, 512] PSUM group is
    the real dot product for batch b.
  - DVE/ScalarE only drain PSUM rows into a [4, 2048] accumulator and
    run a compact log-softmax there: per-group running max, one fused
    exp(x - max) with accum_out for the sum, ln, and a single
    tensor_scalar subtract. One 32KB DMA out.
"""

import numpy as np

S, B, H = 2048, 32, 1024
N_CORES = 8
B_LOC = B // N_CORES          # 4 batches per core
NF = 4                        # s-blocks of 512 (PSUM free-dim limit)
SF = S // NF                  # 512
NHC = 8                       # h-chunks of 128
ENC_COLS = NHC * B_LOC * SF // 2   # free dim of one enc half-tile: 4*4*512

_CACHE = {}


def _build():
    import concourse.bacc as bacc
    import concourse.mybir as mybir
    import concourse.tile as tile
    from concourse.tile import add_dep_helper
    from contextlib import ExitStack

    import os
    dbg = os.environ.get("KDBG", "0") == "1"
    f32 = mybir.dt.float32
    f16 = mybir.dt.float16
    f8 = mybir.dt.float8e3
    nc = bacc.Bacc("TRN2", target_bir_lowering=False, debug=False,
                   num_devices=N_CORES)

    # enc layout: [f*128 + p, (hc, b, s')] with h = hc*128 + p, s = f*512 + s'
    enc = nc.dram_tensor("enc", [NF * 128, NHC * B_LOC * SF], f8,
                         kind="ExternalInput").ap()
    # hidT layout: [p, (oc, b, b')] = hid[b, oc*128+p] iff b == b', else 0.
    # The zero-padding makes the vT matmul emit a block-diagonal masked vT,
    # so every main-loop matmul writes only row b of its PSUM group and all
    # four b's can accumulate into one [4, 512] tile (engine ops must start
    # at partition 0, so per-row drains are illegal).
    hidT = nc.dram_tensor("hidT", [128, 8 * B_LOC * B_LOC], f16,
                          kind="ExternalInput").ap()
    w = nc.dram_tensor("w", [H, H], f16, kind="ExternalInput").ap()
    out = nc.dram_tensor("out", [B_LOC, S], f32, kind="ExternalOutput").ap()
    if dbg:
        vt_dbg = nc.dram_tensor("vt_dbg", [128, NHC * B_LOC * B_LOC], f32,
                                kind="ExternalOutput").ap()
        acc_dbg = nc.dram_tensor("acc_dbg", [B_LOC, S], f32,
                                 kind="ExternalOutput").ap()

    with tile.TileContext(nc) as tc, ExitStack() as ctx:
        const_pool = ctx.enter_context(tc.tile_pool(name="const", bufs=1))
        w_pool = ctx.enter_context(tc.tile_pool(name="wpool", bufs=8))
        enc_pool = ctx.enter_context(tc.tile_pool(name="encp", bufs=8))
        ps_pool = ctx.enter_context(tc.tile_pool(name="ps", bufs=4, space="PSUM"))
        psv_pool = ctx.enter_context(tc.tile_pool(name="psv", bufs=1, space="PSUM"))

        # Preload the exp/ln ACT tables while ScalarE is idle so the
        # epilogue doesn't pay the ~2.6us ACT_TABLE_LOAD cost.
        warm = const_pool.tile([1, 1], f32)
        nc.vector.memset(warm[:], 1.0)
        warm2 = const_pool.tile([1, 1], f32)
        nc.scalar.activation(warm2[:], warm[:], mybir.ActivationFunctionType.Exp)
        nc.scalar.activation(warm2[:], warm2[:], mybir.ActivationFunctionType.Ln)

        # ---- input DMAs ------------------------------------------------
        hidT_sb = const_pool.tile([128, 8 * B_LOC * B_LOC], f16)
        nc.sync.dma_start(hidT_sb[:], hidT[:, :])

        w_tiles = []
        w_dmas = []
        for oc in range(8):
            wt = w_pool.tile([128, H], f16, tag="wt")
            w_dmas.append(nc.sync.dma_start(wt[:], w[oc * 128:(oc + 1) * 128, :]))
            w_tiles.append(wt)

        # enc: 8 DMAs of [128, 4096] (1MB), sequenced behind the W stream
        # so vT (which needs all of W) is ready as early as possible.
        enc_tiles = []  # [f][half]
        for f in range(NF):
            halves = []
            for hf in range(2):
                et = enc_pool.tile([128, ENC_COLS], f8, tag="enc")
                d = nc.sync.dma_start(
                    et[:], enc[f * 128:(f + 1) * 128,
                                hf * ENC_COLS:(hf + 1) * ENC_COLS])
                add_dep_helper(d.ins, w_dmas[7].ins,
                               reason="W stream first: vT gates the main loop")
                halves.append(et)
            enc_tiles.append(halves)

        # ---- vT[h, (b, b')] = sum_o W[o, h] * hidTmask[o, (b, b')] -----
        # lhsT = W chunk [128o, 128h], rhs = hidTmask chunk [128o, 16],
        # accumulated over the 8 o-chunks. Pipelines behind W DMA arrivals.
        # Column group (b, :) of the result is v_b at column b, zeros else.
        # PSUM accumulation groups are bank-scoped: interleaved groups
        # sharing one bank corrupt each other, so each of the 4 concurrent
        # hc-groups gets its own 512-f32 bank and the 8 h-chunks run as
        # two sweeps.
        NB2 = B_LOC * B_LOC
        vT_sb = const_pool.tile([128, NHC * NB2], f8)
        for sweep in range(2):
            psum_vT = psv_pool.tile([128, 4 * 512], f32, tag="vt")
            for oc in range(8):
                for k in range(4):
                    hc = sweep * 4 + k
                    nc.tensor.matmul(
                        psum_vT[:, k * 512:k * 512 + NB2],
                        lhsT=w_tiles[oc][:, hc * 128:(hc + 1) * 128],
                        rhs=hidT_sb[:, oc * NB2:(oc + 1) * NB2],
                        start=(oc == 0), stop=(oc == 7),
                        skip_group_check=True)
            for k in range(4):
                hc = sweep * 4 + k
                nc.vector.tensor_copy(vT_sb[:, hc * NB2:(hc + 1) * NB2],
                                      psum_vT[:, k * 512:k * 512 + NB2])
        if dbg:
            vt_f32 = const_pool.tile([128, NHC * NB2], f32)
            nc.scalar.copy(vt_f32[:], vT_sb[:])
            nc.sync.dma_start(vt_dbg[:, :], vt_f32[:])

        # ---- main loop: energies as PE matmuls -------------------------
        # Per f-block, all 4 b's and all 8 h-chunks accumulate into one
        # [4, 512] PSUM tile; the masked lhsT restricts each matmul to its
        # own output row b.
        acc = const_pool.tile([B_LOC, S], f32)
        mxs = const_pool.tile([B_LOC, NF], f32)
        for f in range(NF):
            ps = ps_pool.tile([B_LOC, SF], f32, tag="mm")
            for b in range(B_LOC):
                for hc in range(NHC):
                    half = enc_tiles[f][hc // 4]
                    col = ((hc % 4) * B_LOC + b) * SF
                    nc.tensor.matmul(
                        ps[:],
                        lhsT=vT_sb[:, hc * NB2 + b * B_LOC:
                                   hc * NB2 + (b + 1) * B_LOC],
                        rhs=half[:, col:col + SF],
                        start=(b == 0 and hc == 0),
                        stop=(b == B_LOC - 1 and hc == NHC - 1),
                        skip_group_check=True)
            nc.scalar.copy(acc[:, f * SF:(f + 1) * SF], ps[:])
            nc.vector.reduce_max(mxs[:, f:f + 1], ps[:],
                                 axis=mybir.AxisListType.X)

        if dbg:
            nc.sync.dma_start(acc_dbg[:, :], acc[:])

        # ---- log_softmax over s in the [4, 2048] layout ----------------
        gmax = const_pool.tile([B_LOC, 1], f32)
        nc.vector.reduce_max(gmax[:], mxs[:], axis=mybir.AxisListType.X)
        ngmax = const_pool.tile([B_LOC, 1], f32)
        nc.vector.tensor_scalar_mul(ngmax[:], gmax[:], -1.0)
        pexp = const_pool.tile([B_LOC, S], f32)
        ssum = const_pool.tile([B_LOC, 1], f32)
        nc.scalar.activation(pexp[:], acc[:], mybir.ActivationFunctionType.Exp,
                             bias=ngmax[:, 0:1], scale=1.0, accum_out=ssum[:])
        lse = const_pool.tile([B_LOC, 1], f32)
        nc.scalar.activation(lse[:], ssum[:], mybir.ActivationFunctionType.Ln)
        ofs = const_pool.tile([B_LOC, 1], f32)
        nc.vector.tensor_tensor(out=ofs[:], in0=gmax[:], in1=lse[:],
                                op=mybir.AluOpType.add)
        final = const_pool.tile([B_LOC, S], f32)
        nc.vector.tensor_scalar(final[:], acc[:], ofs[:, 0:1], None,
                                op0=mybir.AluOpType.subtract)
        nc.sync.dma_start(out[:, :], final[:])

    nc.compile()
    return nc


def _get_nc():
    if "nc" not in _CACHE:
        _CACHE["nc"] = _build()
    return _CACHE["nc"]


def kernel(hidden, encoder_outputs, attn_W, attn_b):
    import ml_dtypes
    from concourse.bass_utils import run_bass_kernel_spmd

    f8 = ml_dtypes.float8_e3m4
    hidden = np.asarray(hidden, dtype=np.float32)
    encoder_outputs = np.asarray(encoder_outputs, dtype=np.float32)
    attn_W = np.ascontiguousarray(np.asarray(attn_W, dtype=np.float16))

    in_maps = []
    for c in range(N_CORES):
        b0 = c * B_LOC
        # enc_t[f, p, hc, b, s'] = enc[f*512+s', b0+b, hc*128+p]
        enc_loc = encoder_outputs[:, b0:b0 + B_LOC, :]          # [S, 4, H]
        enc_t = enc_loc.reshape(NF, SF, B_LOC, NHC, 128)        # [f,s',b,hc,p]
        enc_t = np.ascontiguousarray(enc_t.transpose(0, 4, 3, 2, 1))
        enc_t = enc_t.reshape(NF * 128, NHC * B_LOC * SF).astype(f8)
        # hidT[p, (oc, b, b')] = hid[b, oc*128+p] iff b == b', else 0
        hid_loc = hidden[0, b0:b0 + B_LOC, :]                   # [4, H]
        hidT3 = hid_loc.reshape(B_LOC, 8, 128).transpose(2, 1, 0)  # [p, oc, b]
        hidT = np.zeros((128, 8, B_LOC, B_LOC), dtype=np.float16)
        for b in range(B_LOC):
            hidT[:, :, b, b] = hidT3[:, :, b]
        hidT = hidT.reshape(128, 8 * B_LOC * B_LOC)
        in_maps.append({"enc": enc_t, "hidT": hidT, "w": attn_W})

    nc = _get_nc()
    res = run_bass_kernel_spmd(nc, in_maps, core_ids=list(range(N_CORES)))
    _CACHE["last_results"] = res
    outs = [r["out"] for r in res.results]          # each [B_LOC, S]
    full = np.concatenate(outs, axis=0)             # [B, S]
    return full[:, None, :].astype(np.float32)      # [B, 1, S]


# revision 16
# speedup vs baseline: 1.5027x; 1.0197x over previous
"""Trainium2 Bass kernel for nn_Attn (dense_transformer).

Reference computation:
    proj     = einsum('sbh,oh->sbo', encoder_outputs, attn_W) + attn_b   # [S,B,H]
    energies = einsum('sbh,bh->bs', proj, hidden[0])                     # [B,S]
    out      = log_softmax(energies, axis=-1)[:, None, :]                # [B,1,S]

Algebraic rewrite:
    energies[b,s] = enc[s,b,:] . v[b]  with  v = hidden @ W  (the attn_b
    term is constant per b and cancels inside log_softmax).

Kernel strategy (v3 — TensorE-centric):
  - Data-parallel over batch: 4 b's per core on 8 cores, no collectives.
  - enc is pre-transposed on the host to [h, s] layout and quantized to
    fp8-e3m4 (halves HBM traffic; measured end-to-end rel-err ~1.2e-2 vs
    the 2e-2 gate). W stays fp16 so v keeps near-full precision.
  - The PE computes vT = W^T @ hidT directly in [h, b] layout as one
    64-matmul accumulation group (per-element has_written lets disjoint
    column ranges share one PSUM bank), pipelined behind the W DMA
    chunks. A diagonal strided copy expands vT into the block-masked
    fp8 form the main loop needs.
  - The energies reduction runs as 128 N=512 matmuls with masked vT
    stationary: ps[b', s-block] += vTmask[hc,b]^T @ encT[hc,b,s-block],
    accumulated over 8 h-chunks; the mask confines each b to its own
    PSUM row so all four b's share one [4, 512] tile and drains are
    partition-0-legal.
  - log-softmax is accumulated online: per f-block, ScalarE drains the
    PSUM tile, DVE takes a block max, ScalarE does a fused
    exp(x - m_f) with accum_out. The tail just rescales the four
    partial sums, takes ln, and subtracts via one AP-scalar DVE op.
"""

import numpy as np

S, B, H = 2048, 32, 1024
N_CORES = 8
B_LOC = B // N_CORES          # 4 batches per core
NF = 4                        # s-blocks of 512 (PSUM free-dim limit)
SF = S // NF                  # 512
NHC = 8                       # h-chunks of 128
NQ = 2 * NF                   # 8 enc quarter-DMAs... (2 per f-block)
ENC_COLS = NHC * B_LOC * SF   # free dim of one f-block: 16384

_CACHE = {}


def _build():
    import os
    import concourse.bacc as bacc
    import concourse.mybir as mybir
    import concourse.tile as tile
    from concourse.tile import add_dep_helper
    from contextlib import ExitStack

    dbg = os.environ.get("KDBG", "0") == "1"
    f32 = mybir.dt.float32
    f16 = mybir.dt.float16
    f8 = mybir.dt.float8e3
    nc = bacc.Bacc("TRN2", target_bir_lowering=False, debug=False,
                   num_devices=N_CORES)

    # enc layout: [f*128 + p, (hc, b, s')] with h = hc*128 + p, s = f*512 + s'
    enc = nc.dram_tensor("enc", [NF * 128, ENC_COLS], f8,
                         kind="ExternalInput").ap()
    # hidT layout: [p, (oc, b)] = hid[b, oc*128+p]
    hidT = nc.dram_tensor("hidT", [128, 8 * B_LOC], f16,
                          kind="ExternalInput").ap()
    w = nc.dram_tensor("w", [H, H], f16, kind="ExternalInput").ap()
    out = nc.dram_tensor("out", [B_LOC, S], f32, kind="ExternalOutput").ap()
    if dbg:
        acc_dbg = nc.dram_tensor("acc_dbg", [B_LOC, S], f32,
                                 kind="ExternalOutput").ap()

    with tile.TileContext(nc) as tc, ExitStack() as ctx:
        const_pool = ctx.enter_context(tc.tile_pool(name="const", bufs=1))
        w_pool = ctx.enter_context(tc.tile_pool(name="wpool", bufs=8))
        encq_pool = ctx.enter_context(tc.tile_pool(name="encq", bufs=4))
        ench_pool = ctx.enter_context(tc.tile_pool(name="ench", bufs=6))
        scr_pool = ctx.enter_context(tc.tile_pool(name="scr", bufs=2))
        ps_pool = ctx.enter_context(tc.tile_pool(name="ps", bufs=4, space="PSUM"))
        psv_pool = ctx.enter_context(tc.tile_pool(name="psv", bufs=1, space="PSUM"))

        # Preload the ACT tables while ScalarE is idle. Order matters: the
        # exp set must be the resident one when the per-f-block exps run,
        # so warm Ln first and Exp last.
        warm = const_pool.tile([1, 1], f32)
        nc.vector.memset(warm[:], 1.0)
        warm2 = const_pool.tile([1, 1], f32)
        nc.scalar.activation(warm2[:], warm[:], mybir.ActivationFunctionType.Ln)
        nc.scalar.activation(warm2[:], warm2[:], mybir.ActivationFunctionType.Exp)

        # ---- input DMAs ------------------------------------------------
        hidT_sb = const_pool.tile([128, 8 * B_LOC], f16)
        nc.sync.dma_start(hidT_sb[:], hidT[:, :])

        w_tiles = []
        w_dmas = []
        for oc in range(8):
            wt = w_pool.tile([128, H], f16, tag="wt")
            w_dmas.append(nc.sync.dma_start(wt[:], w[oc * 128:(oc + 1) * 128, :]))
            w_tiles.append(wt)

        # enc: strictly behind the W stream (vT gates the main loop). The
        # first f-block arrives as 4 small quarters so the PE can start as
        # soon as vT is ready; later blocks use 1MB halves.
        enc_tiles = []  # [f] -> list of (tile, col_lo, col_hi)
        for f in range(NF):
            pieces = []
            nsplit = 4 if f == 0 else 2
            cw = ENC_COLS // nsplit
            pool = encq_pool if nsplit == 4 else ench_pool
            for q in range(nsplit):
                et = pool.tile([128, cw], f8, tag=f"enc{nsplit}")
                d = nc.sync.dma_start(
                    et[:], enc[f * 128:(f + 1) * 128, q * cw:(q + 1) * cw])
                add_dep_helper(d.ins, w_dmas[7].ins,
                               reason="W stream first: vT gates the main loop")
                pieces.append((et, q * cw, (q + 1) * cw))
            enc_tiles.append(pieces)

        def enc_rhs(f, hc, b):
            col = (hc * B_LOC + b) * SF
            for et, lo, hi in enc_tiles[f]:
                if lo <= col and col + SF <= hi:
                    return et[:, col - lo:col - lo + SF]
            raise AssertionError("enc slice spans pieces")

        # ---- vT[h, b] = sum_o W[o, h] * hid[b, o] ----------------------
        # lhsT = W chunk [128o, 128h], rhs = hidT chunk [128o, 4b]. One
        # 64-matmul accumulation group: disjoint hc column ranges share a
        # bank via per-element has_written; only the very first matmul
        # clears it. Pipelines behind the W DMA arrivals (oc outer).
        psum_vT = psv_pool.tile([128, NHC * B_LOC], f32)
        for oc in range(8):
            for hc in range(NHC):
                nc.tensor.matmul(
                    psum_vT[:, hc * B_LOC:(hc + 1) * B_LOC],
                    lhsT=w_tiles[oc][:, hc * 128:(hc + 1) * 128],
                    rhs=hidT_sb[:, oc * B_LOC:(oc + 1) * B_LOC],
                    start=(oc == 0 and hc == 0),
                    stop=(oc == 7 and hc == NHC - 1),
                    skip_group_check=True)

        # Masked fp8 form: vTs[p, (hc, b, b')] = vT[p, (hc, b)] iff b == b'.
        # The mask confines each main-loop matmul to PSUM row b, letting
        # all four b's accumulate in one [4, 512] tile with base-0 drains.
        NB2 = B_LOC * B_LOC
        vTs = const_pool.tile([128, NHC * NB2], f8)
        nc.vector.memset(vTs[:], 0.0)
        vTs_v = vTs[:].rearrange("p (hc x) -> p hc x", x=NB2)
        pv_v = psum_vT[:].rearrange("p (hc b) -> p hc b", b=B_LOC)
        for b in range(B_LOC):
            nc.vector.tensor_copy(vTs_v[:, :, b * B_LOC + b:b * B_LOC + b + 1],
                                  pv_v[:, :, b:b + 1])

        # ---- main loop: energies as PE matmuls + online softmax stats --
        acc = const_pool.tile([B_LOC, S], f32)
        mxs = const_pool.tile([B_LOC, NF], f32)
        nmxs = const_pool.tile([B_LOC, NF], f32)
        ssums = const_pool.tile([B_LOC, NF], f32)
        for f in range(NF):
            ps = ps_pool.tile([B_LOC, SF], f32, tag="mm")
            for b in range(B_LOC):
                for hc in range(NHC):
                    nc.tensor.matmul(
                        ps[:],
                        lhsT=vTs[:, hc * NB2 + b * B_LOC:
                                 hc * NB2 + (b + 1) * B_LOC],
                        rhs=enc_rhs(f, hc, b),
                        start=(b == 0 and hc == 0),
                        stop=(b == B_LOC - 1 and hc == NHC - 1),
                        skip_group_check=True)
            nc.scalar.copy(acc[:, f * SF:(f + 1) * SF], ps[:])
            nc.vector.reduce_max(mxs[:, f:f + 1], ps[:],
                                 axis=mybir.AxisListType.X)
            nc.vector.tensor_scalar_mul(nmxs[:, f:f + 1], mxs[:, f:f + 1], -1.0)
            pexp = scr_pool.tile([B_LOC, SF], f32, tag="pexp")
            nc.scalar.activation(pexp[:], acc[:, f * SF:(f + 1) * SF],
                                 mybir.ActivationFunctionType.Exp,
                                 bias=nmxs[:, f:f + 1], scale=1.0,
                                 accum_out=ssums[:, f:f + 1])

        if dbg:
            nc.sync.dma_start(acc_dbg[:, :], acc[:])

        # ---- tail: combine the 4 online blocks -------------------------
        # S_b = sum_f ssums[b,f] * exp(mxs[b,f] - gmax[b]);
        # out = acc - gmax - ln(S)
        gmax = const_pool.tile([B_LOC, 1], f32)
        nc.vector.reduce_max(gmax[:], mxs[:], axis=mybir.AxisListType.X)
        ngmax = const_pool.tile([B_LOC, 1], f32)
        nc.vector.tensor_scalar_mul(ngmax[:], gmax[:], -1.0)
        sc = const_pool.tile([B_LOC, NF], f32)
        nc.scalar.activation(sc[:], mxs[:], mybir.ActivationFunctionType.Exp,
                             bias=ngmax[:, 0:1], scale=1.0)
        wsum = const_pool.tile([B_LOC, NF], f32)
        nc.vector.tensor_tensor(out=wsum[:], in0=ssums[:], in1=sc[:],
                                op=mybir.AluOpType.mult)
        stot = const_pool.tile([B_LOC, 1], f32)
        nc.vector.reduce_sum(stot[:], wsum[:], axis=mybir.AxisListType.X)
        lse = const_pool.tile([B_LOC, 1], f32)
        nc.scalar.activation(lse[:], stot[:], mybir.ActivationFunctionType.Ln)
        ofs = const_pool.tile([B_LOC, 1], f32)
        nc.vector.tensor_tensor(out=ofs[:], in0=gmax[:], in1=lse[:],
                                op=mybir.AluOpType.add)
        final = const_pool.tile([B_LOC, S], f32)
        nc.vector.tensor_scalar(final[:], acc[:], ofs[:, 0:1], None,
                                op0=mybir.AluOpType.subtract)
        nc.sync.dma_start(out[:, :], final[:])

    nc.compile()
    return nc


def _get_nc():
    if "nc" not in _CACHE:
        _CACHE["nc"] = _build()
    return _CACHE["nc"]


def kernel(hidden, encoder_outputs, attn_W, attn_b):
    import ml_dtypes
    from concourse.bass_utils import run_bass_kernel_spmd

    f8 = ml_dtypes.float8_e3m4
    hidden = np.asarray(hidden, dtype=np.float32)
    encoder_outputs = np.asarray(encoder_outputs, dtype=np.float32)
    attn_W = np.ascontiguousarray(np.asarray(attn_W, dtype=np.float16))

    in_maps = []
    for c in range(N_CORES):
        b0 = c * B_LOC
        # enc_t[f, p, hc, b, s'] = enc[f*512+s', b0+b, hc*128+p]
        enc_loc = encoder_outputs[:, b0:b0 + B_LOC, :]          # [S, 4, H]
        enc_t = enc_loc.reshape(NF, SF, B_LOC, NHC, 128)        # [f,s',b,hc,p]
        enc_t = np.ascontiguousarray(enc_t.transpose(0, 4, 3, 2, 1))
        enc_t = enc_t.reshape(NF * 128, ENC_COLS).astype(f8)
        # hidT[p, (oc, b)] = hid[b, oc*128+p]
        hid_loc = hidden[0, b0:b0 + B_LOC, :]                   # [4, H]
        hidT = np.ascontiguousarray(
            hid_loc.reshape(B_LOC, 8, 128).transpose(2, 1, 0)
        ).reshape(128, 8 * B_LOC).astype(np.float16)
        in_maps.append({"enc": enc_t, "hidT": hidT, "w": attn_W})

    nc = _get_nc()
    res = run_bass_kernel_spmd(nc, in_maps, core_ids=list(range(N_CORES)))
    _CACHE["last_results"] = res
    outs = [r["out"] for r in res.results]          # each [B_LOC, S]
    full = np.concatenate(outs, axis=0)             # [B, S]
    return full[:, None, :].astype(np.float32)      # [B, 1, S]
